# revision 25
# baseline (speedup 1.0000x reference)
"""GQA attention kernel for 8 trn2 NeuronCores (Bass/Tile, SPMD).

Problem: X[2,2048,2048] fp32, Wq[2048,2048], Wk/Wv[2048,512], Wo[2048,2048].
  q/k/v proj -> GQA attention (32 Q heads, 8 KV heads, head_dim 64, no mask)
  -> out proj.

Sharding (8 cores): core c handles batch b=c//4 and query heads
[8j, 8j+8) with KV heads {2j, 2j+1} where j=c%4.  Within a core, heads are
processed in pairs (p, p+4): head p (KV head 2j) on partitions 0-63 and
head p+4 (KV head 2j+1) on partitions 64-127.

Transposed layout so all contractions sit on the partition axis:
  proj:    Q^T = Wq_c^T X^T            [fp32r, N=512 tiles]
  scores:  S^T = K Q^T  (row-packed pair of K=64 matmuls, concurrent)
  softmax: P^T = exp(S^T/8) on ScalarE ([128,1024] tiles)
  PV:      O^T = V^T P^T  (col-packed pair of M=64 matmuls, concurrent)
  rowsum:  rs  = 1^T P^T  (two M=1 matmuls at col positions 0/32)
  norm:    recip on DVE (reciprocal_approx_fast), PE ones-broadcast,
           DVE multiply -> bf16
  gather:  per-pair AllGather (bf16) overlapped with later pairs' compute
  oproj:   Y^T = Wo_c^T A^T_full (bf16 rhs), pair-major chunk order
"""

import os
import sys
import types
from contextlib import ExitStack

import numpy as np

_HIDDEN = 2048
_SEQ = 2048
_BATCH = 2
_NCORES = 8
_HD = 64

_KC = _HIDDEN // 128  # 16 contraction chunks for proj/oproj
_NT = _SEQ // 512  # 4 seq tiles of 512
_ST = _SEQ // 128  # 16 key chunks of 128
_NPAIR = 4

_ORDER = [0, 4, 1, 5, 2, 6, 3, 7]  # within-core local head order (pairing)


def _install_ntff_hook():
    """antenv in this image lacks axon_hooks; synthesize it so the axon
    NTFF profiling path works when tracing is requested."""
    try:
        import antenv

        try:
            from antenv import axon_hooks  # noqa: F401

            return
        except ImportError:
            pass
        mod = types.ModuleType("antenv.axon_hooks")
        mod._hook = None
        mod.set_axon_ntff_profile_hook = lambda h: setattr(mod, "_hook", h)
        mod.get_axon_ntff_profile_hook = lambda: mod._hook
        sys.modules["antenv.axon_hooks"] = mod
        antenv.axon_hooks = mod
        from trn_agent_boot.trn_boot import _ntff_profile_via_ctypes

        so = "/opt/axon/libaxon_pjrt.so"
        if os.path.exists(so):
            mod.set_axon_ntff_profile_hook(_ntff_profile_via_ctypes(so))
    except Exception:
        pass


_install_ntff_hook()

import concourse.bass as bass  # noqa: E402
import concourse.tile as tile  # noqa: E402
from concourse import bacc, bass_utils, mybir  # noqa: E402
from concourse.bass_utils import run_bass_kernel_spmd  # noqa: E402
from concourse.masks import make_identity  # noqa: E402

bass_utils.upload_artifacts = lambda tmpdir: tmpdir

F32 = mybir.dt.float32
F32R = mybir.dt.float32r
BF16 = mybir.dt.bfloat16

_nc_cache = None
_last_results = None


def _build():
    nc = bacc.Bacc("TRN2", target_bir_lowering=False, debug=False, num_devices=8)

    xt_d = nc.declare_dram_parameter("xt", [_HIDDEN, _SEQ], F32, isOutput=False)
    wq_d = nc.declare_dram_parameter("wq", [_HIDDEN, 512], F32, isOutput=False)
    wk_d = nc.declare_dram_parameter("wk", [_HIDDEN, 128], F32, isOutput=False)
    wv_d = nc.declare_dram_parameter("wv", [_HIDDEN, 128], F32, isOutput=False)
    wo_d = nc.declare_dram_parameter("wo", [_HIDDEN, 512], F32, isOutput=False)
    ones_d = nc.declare_dram_parameter("ones", [128, _ST], F32, isOutput=False)
    yt_d = nc.declare_dram_parameter("yt", [512, _SEQ], F32, isOutput=True)

    at_loc = [nc.dram_tensor(f"at_loc{p}", [128, _SEQ], BF16) for p in range(_NPAIR)]
    at_full = [
        nc.dram_tensor(f"at_full{p}", [512, _SEQ], BF16) for p in range(_NPAIR)
    ]

    with (
        tile.TileContext(nc) as tc,
        ExitStack() as ctx,
        nc.allow_low_precision(reason="fp32r/bf16 within 2e-2 tolerance"),
    ):
        const = ctx.enter_context(tc.tile_pool(name="const", bufs=1))

        # ---- persistent SBUF tensors -------------------------------------
        wq_sb = const.tile([128, _KC, 512], F32R, tag="wq")
        wk_sb = const.tile([128, _KC, 128], F32R, tag="wk")
        wv_sb = const.tile([128, _KC, 128], F32R, tag="wv")
        wo_sb = const.tile([128, _KC, 512], BF16, tag="wo")
        nc.sync.dma_start(
            out=wq_sb, in_=wq_d[:, :].rearrange("(kc p) m -> p kc m", p=128).bitcast(F32R)
        )
        nc.sync.dma_start(
            out=wk_sb, in_=wk_d[:, :].rearrange("(kc p) m -> p kc m", p=128).bitcast(F32R)
        )
        nc.sync.dma_start(
            out=wv_sb, in_=wv_d[:, :].rearrange("(kc p) m -> p kc m", p=128).bitcast(F32R)
        )

        ident = const.tile([128, 128], F32, tag="ident")
        make_identity(nc, ident)
        # ones row at partition 64 for the recip broadcast matmuls (K=1)
        ones_row = const.tile([65, 64], F32, tag="ones_row")
        nc.vector.memset(ones_row, 1.0)

        qt_sb = [
            const.tile([128, _SEQ], F32R, tag=f"qt{p}", name=f"qt{p}")
            for p in range(_NPAIR)
        ]
        kt_sb = const.tile([128, _SEQ], F32R, tag="kt")
        # [V_A | 1 | V_B | 1] per key chunk (fused rowsum rows at 64/129)
        vone = const.tile([128, _ST, 130], F32R, tag="vone")
        nc.sync.dma_start(out=vone[:, :, 64], in_=ones_d[:, :].bitcast(F32R))
        nc.sync.dma_start(out=vone[:, :, 129], in_=ones_d[:, :].bitcast(F32R))

        # ---- phase 1: q/k/v projections ---------------------------------
        with (
            tc.tile_pool(name="xt_pool", bufs=8) as xt_pool,
            tc.tile_pool(name="vt_pool", bufs=2) as vt_pool,
            tc.tile_pool(name="wstage", bufs=1) as wstage,
            tc.tile_pool(name="p_pool", bufs=4) as p_pool,
            tc.tile_pool(name="rr_pool", bufs=2) as rr_pool,
            tc.tile_pool(name="oraw_pool", bufs=2) as oraw_pool,
            tc.tile_pool(name="rbsb_pool", bufs=2) as rbsb_pool,
            tc.tile_pool(name="at_pool", bufs=2) as at_pool,
        ):
            # ---- phase 1a: k/v projection (all seq tiles) ---------------
            with (
                tc.tile_pool(name="kv_ps", bufs=1, space="PSUM") as kv_ps,
                tc.tile_pool(name="tp_ps", bufs=2, space="PSUM") as tp_ps,
            ):
                for n in range(_NT):
                    psk = kv_ps.tile([128, 512], F32, tag="psk")
                    psv = kv_ps.tile([128, 512], F32, tag="psv")
                    for kc in range(_KC):
                        xt_t = xt_pool.tile([128, 512], F32R, tag="xt")
                        nc.sync.dma_start(
                            out=xt_t,
                            in_=xt_d[
                                kc * 128 : (kc + 1) * 128, n * 512 : (n + 1) * 512
                            ].bitcast(F32R),
                        )
                        st, sp = kc == 0, kc == _KC - 1
                        nc.tensor.matmul(psk, wk_sb[:, kc, :], xt_t, start=st, stop=sp)
                        nc.tensor.matmul(psv, wv_sb[:, kc, :], xt_t, start=st, stop=sp)
                    nsl = slice(n * 512, (n + 1) * 512)
                    nc.scalar.copy(kt_sb[:, nsl], psk)
                    vt_t = vt_pool.tile([128, 512], F32, tag="vt")
                    nc.scalar.copy(vt_t, psv)
                    for t in range(4):
                        sc = n * 4 + t
                        tp = tp_ps.tile([128, 128], F32, tag="tp")
                        nc.tensor.transpose(
                            tp, vt_t[:, t * 128 : (t + 1) * 128], ident
                        )
                        nc.vector.tensor_copy(vone[:, sc, 0:64], tp[:, 0:64])
                        nc.vector.tensor_copy(vone[:, sc, 65:129], tp[:, 64:128])

            # ---- phase 1b/2: q proj per pair-half, fused with attention -
            with (
                tc.tile_pool(name="q_ps", bufs=1, space="PSUM") as q_ps,
                tc.tile_pool(name="sc_ps", bufs=2, space="PSUM") as sc_ps,
                tc.tile_pool(name="o_ps", bufs=1, space="PSUM") as o_ps,
            ):
                def emit_qproj(pair_lo):
                    # project Q for pairs {pair_lo, pair_lo+1}, all seq tiles
                    for n in range(_NT):
                        psq = [
                            q_ps.tile([128, 512], F32, tag=f"psq{i}", name=f"psq{i}")
                            for i in range(2)
                        ]
                        for kc in range(_KC):
                            xt_t = xt_pool.tile([128, 512], F32R, tag="xt")
                            nc.sync.dma_start(
                                out=xt_t,
                                in_=xt_d[
                                    kc * 128 : (kc + 1) * 128,
                                    n * 512 : (n + 1) * 512,
                                ].bitcast(F32R),
                            )
                            st, sp = kc == 0, kc == _KC - 1
                            for i in range(2):
                                m = pair_lo + i
                                nc.tensor.matmul(
                                    psq[i],
                                    wq_sb[:, kc, m * 128 : (m + 1) * 128],
                                    xt_t,
                                    start=st,
                                    stop=sp,
                                )
                        nsl = slice(n * 512, (n + 1) * 512)
                        for i in range(2):
                            nc.scalar.copy(qt_sb[pair_lo + i][:, nsl], psq[i])

                # wo load + bf16 convert on otherwise-idle DMA/DVE capacity
                for h in range(2):
                    wo_stage = wstage.tile([128, _KC // 2, 512], F32, tag="wostage")
                    nc.sync.dma_start(
                        out=wo_stage,
                        in_=wo_d[h * 1024 : (h + 1) * 1024, :].rearrange(
                            "(kc p) m -> p kc m", p=128
                        ),
                    )
                    nc.vector.tensor_copy(wo_sb[:, h * 8 : (h + 1) * 8, :], wo_stage)

                deferred = None

                def emit_deferred(d):
                    # GpSimd broadcast of reciprocals + normalize + store;
                    # emitted one tile late so these never stall the next
                    # tile's kt loop.
                    p_, n_, rr_, oraw_ = d
                    rb_sb = rbsb_pool.tile([64, 1024], F32, tag="rbsb")
                    nc.gpsimd.partition_broadcast(rb_sb, rr_)
                    at_t = at_pool.tile([64, 1024], BF16, tag="at")
                    nc.vector.tensor_mul(at_t, oraw_[0:64, :], rb_sb)
                    nsl_ = slice(n_ * 512, (n_ + 1) * 512)
                    nc.sync.dma_start(
                        out=at_loc[p_][0:64, nsl_], in_=at_t[:, 0:512]
                    )
                    nc.sync.dma_start(
                        out=at_loc[p_][64:128, nsl_], in_=at_t[:, 512:1024]
                    )

                def emit_att_tile(p, n):
                    nonlocal deferred
                    nsl = slice(n * 512, (n + 1) * 512)
                    o_t = o_ps.tile([128, 1024], F32, tag="o")
                    for kt in range(_ST):
                        ksl = slice(kt * 128, (kt + 1) * 128)
                        s_pair = sc_ps.tile([128, 1024], F32, tag="s_pair")
                        nc.tensor.matmul(
                            s_pair[:, 0:512],
                            kt_sb[0:64, ksl],
                            qt_sb[p][0:64, nsl],
                            start=True,
                            stop=True,
                            tile_position=(0, 0),
                        )
                        nc.tensor.matmul(
                            s_pair[:, 512:1024],
                            kt_sb[64:128, ksl],
                            qt_sb[p][64:128, nsl],
                            start=True,
                            stop=True,
                            tile_position=(64, 0),
                        )
                        pp = p_pool.tile([128, 1024], F32R, tag="pp")
                        nc.scalar.activation(
                            pp, s_pair, mybir.ActivationFunctionType.Exp, scale=0.125
                        )
                        st, sp = kt == 0, kt == _ST - 1
                        nc.tensor.matmul(
                            o_t[0:65, 0:512], vone[:, kt, 0:65], pp[:, 0:512],
                            start=st, stop=sp,
                        )
                        nc.tensor.matmul(
                            o_t[0:65, 512:1024], vone[:, kt, 65:130],
                            pp[:, 512:1024], start=st, stop=sp,
                        )
                    # early tail: free PSUM quickly (DVE only)
                    oraw = oraw_pool.tile([65, 1024], F32, tag="oraw")
                    nc.vector.tensor_copy(oraw, o_t[0:65, :])
                    rr = rr_pool.tile([1, 1024], F32, tag="rr")
                    nc.vector.reciprocal(rr, oraw[64:65, :])
                    if deferred is not None:
                        dp, dn = deferred[0], deferred[1]
                        emit_deferred(deferred)
                        if dn == _NT - 1:
                            # pair dp's last store emitted -> gather it
                            nc.gpsimd.collective_compute(
                                "AllGather",
                                mybir.AluOpType.bypass,
                                replica_groups=[[0, 1, 2, 3], [4, 5, 6, 7]],
                                ins=[at_loc[dp][:, :]],
                                outs=[at_full[dp][:, :]],
                            )
                    deferred = (p, n, rr, oraw)

                for half in range(2):
                    emit_qproj(2 * half)
                    for p in (2 * half, 2 * half + 1):
                        for n in range(_NT):
                            emit_att_tile(p, n)
                emit_deferred(deferred)
                nc.gpsimd.collective_compute(
                    "AllGather",
                    mybir.AluOpType.bypass,
                    replica_groups=[[0, 1, 2, 3], [4, 5, 6, 7]],
                    ins=[at_loc[_NPAIR - 1][:, :]],
                    outs=[at_full[_NPAIR - 1][:, :]],
                )

        # ---- phase 4: output projection (Y^T = Wo_c^T @ A^T_full) -------
        with (
            tc.tile_pool(name="ac_pool", bufs=6) as ac_pool,
            tc.tile_pool(name="y_pool", bufs=2) as y_pool,
            tc.tile_pool(name="y_ps", bufs=2, space="PSUM") as y_ps,
        ):
            for n in range(_NT):
                nsl = slice(n * 512, (n + 1) * 512)
                psy = [
                    y_ps.tile([128, 512], F32, tag=f"psy{m}", name=f"psy{m}")
                    for m in range(4)
                ]
                for p in range(_NPAIR):
                    for jj in range(4):
                        kc = p * 4 + jj
                        ac_t = ac_pool.tile([128, 512], BF16, tag="ac")
                        nc.sync.dma_start(
                            out=ac_t,
                            in_=at_full[p][jj * 128 : (jj + 1) * 128, nsl],
                        )
                        st, sp = kc == 0, kc == _KC - 1
                        for m in range(4):
                            nc.tensor.matmul(
                                psy[m],
                                wo_sb[:, kc, m * 128 : (m + 1) * 128],
                                ac_t,
                                start=st,
                                stop=sp,
                            )
                for m in range(4):
                    y_sb = y_pool.tile([128, 512], F32, tag="y")
                    nc.scalar.copy(y_sb, psy[m])
                    nc.sync.dma_start(
                        out=yt_d[m * 128 : (m + 1) * 128, nsl], in_=y_sb
                    )

    nc.compile()
    return nc


def kernel(X, Wq, Wk, Wv, Wo):
    global _nc_cache, _last_results
    X = np.ascontiguousarray(np.asarray(X, dtype=np.float32))
    Wq = np.asarray(Wq, dtype=np.float32)
    Wk = np.asarray(Wk, dtype=np.float32)
    Wv = np.asarray(Wv, dtype=np.float32)
    Wo = np.asarray(Wo, dtype=np.float32)

    if _nc_cache is None:
        _nc_cache = _build()
    nc = _nc_cache

    xts = [np.ascontiguousarray(X[b].T) for b in range(_BATCH)]
    # Wo rows in gathered order: for pair p, for group-core jj: head(8jj+p)
    # dims then head(8jj+p+4) dims
    perm_rows = []
    for p in range(_NPAIR):
        for jj in range(4):
            for o in (p, p + 4):
                h = 8 * jj + o
                perm_rows.extend(range(h * _HD, (h + 1) * _HD))
    wo_p = Wo[perm_rows, :]

    in_maps = []
    for c in range(_NCORES):
        b, j = divmod(c, 4)
        qcols = []
        for o in _ORDER:
            h = 8 * j + o
            qcols.extend(range(h * _HD, (h + 1) * _HD))
        in_maps.append(
            {
                "xt": xts[b],
                "wq": np.ascontiguousarray(Wq[:, qcols]),
                "wk": np.ascontiguousarray(Wk[:, 2 * j * _HD : (2 * j + 2) * _HD]),
                "wv": np.ascontiguousarray(Wv[:, 2 * j * _HD : (2 * j + 2) * _HD]),
                "wo": np.ascontiguousarray(wo_p[:, 512 * j : 512 * (j + 1)]),
                "ones": np.ones((128, _ST), dtype=np.float32),
            }
        )

    trace = bool(os.environ.get("KERNEL_TRACE"))
    res = run_bass_kernel_spmd(
        nc, in_maps, core_ids=list(range(_NCORES)), trace=trace
    )
    _last_results = res

    Y = np.empty((_BATCH, _SEQ, _HIDDEN), dtype=np.float32)
    for c in range(_NCORES):
        b, j = divmod(c, 4)
        Y[b][:, 512 * j : 512 * (j + 1)] = res.results[c]["yt"].T
    return Y


# revision 26
# speedup vs baseline: 1.1925x; 1.1925x over previous
"""GQA attention kernel for 8 trn2 NeuronCores (Bass/Tile, SPMD).

Problem: X[2,2048,2048] fp32, Wq[2048,2048], Wk/Wv[2048,512], Wo[2048,2048].
  q/k/v proj -> GQA attention (32 Q heads, 8 KV heads, head_dim 64, no mask)
  -> out proj.

Sharding (8 cores): core c handles batch b=c//4 and query heads
[8j, 8j+8) with KV heads {2j, 2j+1} where j=c%4.  Within a core, heads are
processed in pairs (p, p+4): head p (KV head 2j) on partitions 0-63 and
head p+4 (KV head 2j+1) on partitions 64-127.

Transposed layout so all contractions sit on the partition axis:
  proj:    Q^T = Wq_c^T X^T            [fp32r, N=512 tiles]
  scores:  S^T = K Q^T  (row-packed pair of K=64 matmuls, concurrent)
  softmax: P^T = exp(S^T/8) on ScalarE ([128,1024] tiles)
  PV:      O^T = V^T P^T  (col-packed pair of M=64 matmuls, concurrent)
  rowsum:  rs  = 1^T P^T  (two M=1 matmuls at col positions 0/32)
  norm:    recip on DVE (reciprocal_approx_fast), PE ones-broadcast,
           DVE multiply -> bf16
  gather:  per-pair AllGather (bf16) overlapped with later pairs' compute
  oproj:   Y^T = Wo_c^T A^T_full (bf16 rhs), pair-major chunk order
"""

import os
import sys
import types
from contextlib import ExitStack

import numpy as np

_HIDDEN = 2048
_SEQ = 2048
_BATCH = 2
_NCORES = 8
_HD = 64

_KC = _HIDDEN // 128  # 16 contraction chunks for proj/oproj
_NT = _SEQ // 512  # 4 seq tiles of 512
_ST = _SEQ // 128  # 16 key chunks of 128
_NPAIR = 4

_ORDER = [0, 4, 1, 5, 2, 6, 3, 7]  # within-core local head order (pairing)


def _install_ntff_hook():
    """antenv in this image lacks axon_hooks; synthesize it so the axon
    NTFF profiling path works when tracing is requested."""
    try:
        import antenv

        try:
            from antenv import axon_hooks  # noqa: F401

            return
        except ImportError:
            pass
        mod = types.ModuleType("antenv.axon_hooks")
        mod._hook = None
        mod.set_axon_ntff_profile_hook = lambda h: setattr(mod, "_hook", h)
        mod.get_axon_ntff_profile_hook = lambda: mod._hook
        sys.modules["antenv.axon_hooks"] = mod
        antenv.axon_hooks = mod
        from trn_agent_boot.trn_boot import _ntff_profile_via_ctypes

        so = "/opt/axon/libaxon_pjrt.so"
        if os.path.exists(so):
            mod.set_axon_ntff_profile_hook(_ntff_profile_via_ctypes(so))
    except Exception:
        pass


_install_ntff_hook()

import concourse.bass as bass  # noqa: E402
import concourse.tile as tile  # noqa: E402
from concourse import bacc, bass_utils, mybir  # noqa: E402
from concourse.bass_utils import run_bass_kernel_spmd  # noqa: E402
from concourse.masks import make_identity  # noqa: E402

bass_utils.upload_artifacts = lambda tmpdir: tmpdir

F32 = mybir.dt.float32
F32R = mybir.dt.float32r
BF16 = mybir.dt.bfloat16

_nc_cache = None
_last_results = None


def _build():
    nc = bacc.Bacc("TRN2", target_bir_lowering=False, debug=False, num_devices=8)

    xt_d = nc.declare_dram_parameter("xt", [_HIDDEN, _SEQ], BF16, isOutput=False)
    wq_d = nc.declare_dram_parameter("wq", [_HIDDEN, 512], BF16, isOutput=False)
    wk_d = nc.declare_dram_parameter("wk", [_HIDDEN, 128], BF16, isOutput=False)
    wv_d = nc.declare_dram_parameter("wv", [_HIDDEN, 128], BF16, isOutput=False)
    wo_d = nc.declare_dram_parameter("wo", [_HIDDEN, 512], F32, isOutput=False)
    ones_d = nc.declare_dram_parameter("ones", [128, _ST], BF16, isOutput=False)
    yt_d = nc.declare_dram_parameter("yt", [512, _SEQ], F32, isOutput=True)

    at_loc = [nc.dram_tensor(f"at_loc{p}", [128, _SEQ], BF16) for p in range(_NPAIR)]
    at_full = [
        nc.dram_tensor(f"at_full{p}", [512, _SEQ], BF16) for p in range(_NPAIR)
    ]

    with (
        tile.TileContext(nc) as tc,
        ExitStack() as ctx,
        nc.allow_low_precision(reason="fp32r/bf16 within 2e-2 tolerance"),
    ):
        const = ctx.enter_context(tc.tile_pool(name="const", bufs=1))

        # ---- persistent SBUF tensors -------------------------------------
        wq_sb = const.tile([128, _KC, 512], BF16, tag="wq")
        wk_sb = const.tile([128, _KC, 128], BF16, tag="wk")
        wv_sb = const.tile([128, _KC, 128], BF16, tag="wv")
        wo_sb = const.tile([128, _KC, 512], BF16, tag="wo")
        nc.sync.dma_start(
            out=wq_sb, in_=wq_d[:, :].rearrange("(kc p) m -> p kc m", p=128)
        )
        nc.sync.dma_start(
            out=wk_sb, in_=wk_d[:, :].rearrange("(kc p) m -> p kc m", p=128)
        )
        nc.sync.dma_start(
            out=wv_sb, in_=wv_d[:, :].rearrange("(kc p) m -> p kc m", p=128)
        )

        ident = const.tile([128, 128], F32, tag="ident")
        make_identity(nc, ident)
        # ones row at partition 64 for the recip broadcast matmuls (K=1)
        ones_row = const.tile([65, 64], F32, tag="ones_row")
        nc.vector.memset(ones_row, 1.0)

        qt_sb = [
            const.tile([128, _SEQ], BF16, tag=f"qt{p}", name=f"qt{p}")
            for p in range(_NPAIR)
        ]
        kt_sb = const.tile([128, _SEQ], BF16, tag="kt")
        # [V_A | 1 | V_B | 1] per key chunk (fused rowsum rows at 64/129)
        vone = const.tile([128, _ST, 130], BF16, tag="vone")
        nc.sync.dma_start(out=vone[:, :, 64], in_=ones_d[:, :])
        nc.sync.dma_start(out=vone[:, :, 129], in_=ones_d[:, :])

        # ---- phase 1: q/k/v projections ---------------------------------
        with (
            tc.tile_pool(name="xt_pool", bufs=8) as xt_pool,
            tc.tile_pool(name="vt_pool", bufs=2) as vt_pool,
            tc.tile_pool(name="proj_ps", bufs=1, space="PSUM") as proj_ps,
            tc.tile_pool(name="tp_ps", bufs=2, space="PSUM") as tp_ps,
        ):
            for n in range(_NT):
                psq = [
                    proj_ps.tile([128, 512], F32, tag=f"psq{m}", name=f"psq{m}")
                    for m in range(4)
                ]
                psk = proj_ps.tile([128, 512], F32, tag="psk")
                psv = proj_ps.tile([128, 512], F32, tag="psv")
                for kc in range(_KC):
                    xt_t = xt_pool.tile([128, 512], BF16, tag="xt")
                    nc.sync.dma_start(
                        out=xt_t,
                        in_=xt_d[
                            kc * 128 : (kc + 1) * 128, n * 512 : (n + 1) * 512
                        ],
                    )
                    st, sp = kc == 0, kc == _KC - 1
                    for m in range(4):
                        nc.tensor.matmul(
                            psq[m],
                            wq_sb[:, kc, m * 128 : (m + 1) * 128],
                            xt_t,
                            start=st,
                            stop=sp,
                        )
                    nc.tensor.matmul(psk, wk_sb[:, kc, :], xt_t, start=st, stop=sp)
                    nc.tensor.matmul(psv, wv_sb[:, kc, :], xt_t, start=st, stop=sp)
                nsl = slice(n * 512, (n + 1) * 512)
                for m in range(4):
                    nc.scalar.copy(qt_sb[m][:, nsl], psq[m])
                nc.scalar.copy(kt_sb[:, nsl], psk)
                vt_t = vt_pool.tile([128, 512], F32, tag="vt")
                nc.scalar.copy(vt_t, psv)
                for t in range(4):
                    sc = n * 4 + t
                    tp = tp_ps.tile([128, 128], F32, tag="tp")
                    nc.tensor.transpose(tp, vt_t[:, t * 128 : (t + 1) * 128], ident)
                    nc.vector.tensor_copy(vone[:, sc, 0:64], tp[:, 0:64])
                    nc.vector.tensor_copy(vone[:, sc, 65:129], tp[:, 64:128])
        # ---- phase 2: attention -----------------------------------------
        with (
            tc.tile_pool(name="wstage", bufs=1) as wstage,
            tc.tile_pool(name="p_pool", bufs=4) as p_pool,
            tc.tile_pool(name="rr_pool", bufs=2) as rr_pool,
            tc.tile_pool(name="oraw_pool", bufs=2) as oraw_pool,
            tc.tile_pool(name="rbsb_pool", bufs=2) as rbsb_pool,
            tc.tile_pool(name="at_pool", bufs=2) as at_pool,
            tc.tile_pool(name="sc_ps", bufs=3, space="PSUM") as sc_ps,
            tc.tile_pool(name="o_ps", bufs=1, space="PSUM") as o_ps,
        ):
            # wo load + bf16 convert on otherwise-idle DMA/DVE capacity
            wo_stage = wstage.tile([128, _KC, 512], F32, tag="wostage")
            nc.sync.dma_start(
                out=wo_stage, in_=wo_d[:, :].rearrange("(kc p) m -> p kc m", p=128)
            )
            nc.vector.tensor_copy(wo_sb, wo_stage)

            deferred = None

            def emit_deferred(d):
                # GpSimd broadcast of reciprocals + normalize + store; emitted
                # one tile late so these never stall the next tile's kt loop.
                p_, n_, rr_, oraw_ = d
                rb_sb = rbsb_pool.tile([64, 1024], F32, tag="rbsb")
                nc.gpsimd.partition_broadcast(rb_sb, rr_)
                at_t = at_pool.tile([64, 1024], BF16, tag="at")
                nc.vector.tensor_mul(at_t, oraw_[0:64, :], rb_sb)
                nsl_ = slice(n_ * 512, (n_ + 1) * 512)
                nc.sync.dma_start(out=at_loc[p_][0:64, nsl_], in_=at_t[:, 0:512])
                nc.sync.dma_start(
                    out=at_loc[p_][64:128, nsl_], in_=at_t[:, 512:1024]
                )

            for p in range(_NPAIR):
                for n in range(_NT):
                    nsl = slice(n * 512, (n + 1) * 512)
                    o_t = o_ps.tile([128, 1024], F32, tag="o")
                    for kt in range(_ST):
                        ksl = slice(kt * 128, (kt + 1) * 128)
                        s_pair = sc_ps.tile([128, 1024], F32, tag="s_pair")
                        nc.tensor.matmul(
                            s_pair[:, 0:512],
                            kt_sb[0:64, ksl],
                            qt_sb[p][0:64, nsl],
                            start=True,
                            stop=True,
                            tile_position=(0, 0),
                        )
                        nc.tensor.matmul(
                            s_pair[:, 512:1024],
                            kt_sb[64:128, ksl],
                            qt_sb[p][64:128, nsl],
                            start=True,
                            stop=True,
                            tile_position=(64, 0),
                        )
                        pp = p_pool.tile([128, 1024], BF16, tag="pp")
                        nc.scalar.activation(
                            pp, s_pair, mybir.ActivationFunctionType.Exp, scale=0.125
                        )
                        st, sp = kt == 0, kt == _ST - 1
                        nc.tensor.matmul(
                            o_t[0:65, 0:512], vone[:, kt, 0:65], pp[:, 0:512],
                            start=st, stop=sp,
                        )
                        nc.tensor.matmul(
                            o_t[0:65, 512:1024], vone[:, kt, 65:130],
                            pp[:, 512:1024], start=st, stop=sp,
                        )
                    # early tail: free PSUM quickly (DVE only)
                    oraw = oraw_pool.tile([65, 1024], F32, tag="oraw")
                    nc.vector.tensor_copy(oraw, o_t[0:65, :])
                    rr = rr_pool.tile([1, 1024], F32, tag="rr")
                    nc.vector.reciprocal(rr, oraw[64:65, :])
                    if deferred is not None:
                        dp, dn = deferred[0], deferred[1]
                        emit_deferred(deferred)
                        if dn == _NT - 1:
                            # pair dp's last store emitted -> gather it
                            nc.gpsimd.collective_compute(
                                "AllGather",
                                mybir.AluOpType.bypass,
                                replica_groups=[[0, 1, 2, 3], [4, 5, 6, 7]],
                                ins=[at_loc[dp][:, :]],
                                outs=[at_full[dp][:, :]],
                            )
                    deferred = (p, n, rr, oraw)
            emit_deferred(deferred)
            nc.gpsimd.collective_compute(
                "AllGather",
                mybir.AluOpType.bypass,
                replica_groups=[[0, 1, 2, 3], [4, 5, 6, 7]],
                ins=[at_loc[_NPAIR - 1][:, :]],
                outs=[at_full[_NPAIR - 1][:, :]],
            )

        # ---- phase 4: output projection (Y^T = Wo_c^T @ A^T_full) -------
        with (
            tc.tile_pool(name="ac_pool", bufs=6) as ac_pool,
            tc.tile_pool(name="y_pool", bufs=2) as y_pool,
            tc.tile_pool(name="y_ps", bufs=2, space="PSUM") as y_ps,
        ):
            for n in range(_NT):
                nsl = slice(n * 512, (n + 1) * 512)
                psy = [
                    y_ps.tile([128, 512], F32, tag=f"psy{m}", name=f"psy{m}")
                    for m in range(4)
                ]
                for p in range(_NPAIR):
                    for jj in range(4):
                        kc = p * 4 + jj
                        ac_t = ac_pool.tile([128, 512], BF16, tag="ac")
                        nc.sync.dma_start(
                            out=ac_t,
                            in_=at_full[p][jj * 128 : (jj + 1) * 128, nsl],
                        )
                        st, sp = kc == 0, kc == _KC - 1
                        for m in range(4):
                            nc.tensor.matmul(
                                psy[m],
                                wo_sb[:, kc, m * 128 : (m + 1) * 128],
                                ac_t,
                                start=st,
                                stop=sp,
                            )
                for m in range(4):
                    y_sb = y_pool.tile([128, 512], F32, tag="y")
                    nc.scalar.copy(y_sb, psy[m])
                    nc.sync.dma_start(
                        out=yt_d[m * 128 : (m + 1) * 128, nsl], in_=y_sb
                    )

    nc.compile()
    return nc


def kernel(X, Wq, Wk, Wv, Wo):
    global _nc_cache, _last_results
    X = np.ascontiguousarray(np.asarray(X, dtype=np.float32))
    Wq = np.asarray(Wq, dtype=np.float32)
    Wk = np.asarray(Wk, dtype=np.float32)
    Wv = np.asarray(Wv, dtype=np.float32)
    Wo = np.asarray(Wo, dtype=np.float32)

    if _nc_cache is None:
        _nc_cache = _build()
    nc = _nc_cache

    from ml_dtypes import bfloat16 as _bf16

    xts = [np.ascontiguousarray(X[b].T.astype(_bf16)) for b in range(_BATCH)]
    # Wo rows in gathered order: for pair p, for group-core jj: head(8jj+p)
    # dims then head(8jj+p+4) dims
    perm_rows = []
    for p in range(_NPAIR):
        for jj in range(4):
            for o in (p, p + 4):
                h = 8 * jj + o
                perm_rows.extend(range(h * _HD, (h + 1) * _HD))
    wo_p = Wo[perm_rows, :]

    in_maps = []
    for c in range(_NCORES):
        b, j = divmod(c, 4)
        qcols = []
        for o in _ORDER:
            h = 8 * j + o
            qcols.extend(range(h * _HD, (h + 1) * _HD))
        in_maps.append(
            {
                "xt": xts[b],
                "wq": np.ascontiguousarray(Wq[:, qcols].astype(_bf16)),
                "wk": np.ascontiguousarray(Wk[:, 2 * j * _HD : (2 * j + 2) * _HD].astype(_bf16)),
                "wv": np.ascontiguousarray(Wv[:, 2 * j * _HD : (2 * j + 2) * _HD].astype(_bf16)),
                "wo": np.ascontiguousarray(wo_p[:, 512 * j : 512 * (j + 1)]),
                "ones": np.ones((128, _ST), dtype=_bf16),
            }
        )

    trace = bool(os.environ.get("KERNEL_TRACE"))
    res = run_bass_kernel_spmd(
        nc, in_maps, core_ids=list(range(_NCORES)), trace=trace
    )
    _last_results = res

    Y = np.empty((_BATCH, _SEQ, _HIDDEN), dtype=np.float32)
    for c in range(_NCORES):
        b, j = divmod(c, 4)
        Y[b][:, 512 * j : 512 * (j + 1)] = res.results[c]["yt"].T
    return Y
